# revision 3
# baseline (speedup 1.0000x reference)
"""Trainium2 Bass kernel for nn_AttentionNeNode (8-core SPMD).

Math being computed (see problem reference):
    sel  = inputs[:, in_idxs]            # [R, L] column gather
    qkv  = sel @ weights                 # [R, 3] -> q, k, v columns
    out  = sigmoid(softmax(q[-1] * k.T) @ v)   # only the LAST row's attention matters

Key transformations:
  1. Column gather + matmul == dense matmul with scattered weights:
         sel @ weights == inputs @ W_dense,
     where W_dense[f] = sum of weights[l] over l with in_idxs[l] == f.
     This turns random column access into a dense streaming read of `inputs`.
  2. Only row R-1's attention is needed, so each core computes k, v for its
     block of rows plus flash-softmax partial stats (max, sum_exp, sum_exp*v);
     host combines 8 triples of scalars (the "unshard" step).
  3. `inputs` is pre-transposed/tiled on host so the contraction dim (F) lands
     on SBUF partitions and every DMA is a fully contiguous 2 MiB block.
"""

import sys

if "/opt/trn_rl_repo" not in sys.path:
    sys.path.insert(0, "/opt/trn_rl_repo")

import numpy as np

import concourse.bacc as bacc
import concourse.tile as tile
from concourse import mybir
from concourse.bass_utils import run_bass_kernel_spmd

R, F = 8192, 4096
NCORES = 8
RB = R // NCORES            # 1024 rows per core
NSLICE = 2                  # row slices per core (fit one PSUM bank each for k and v)
SLICE = RB // NSLICE        # 512
NCHUNK = F // 128           # 32 contraction chunks of 128
CH_PER_TILE = 8             # f-chunks per DMA tile
NTILE = NCHUNK // CH_PER_TILE * NSLICE  # 8 DMA tiles of [128, 4096] = 2 MiB
F32 = mybir.dt.float32

_NC = None


def _build_nc():
    nc = bacc.Bacc("TRN2", target_bir_lowering=False, debug=False)
    xt = nc.dram_tensor("xt", [NTILE, 128, CH_PER_TILE * SLICE], F32,
                        kind="ExternalInput").ap()
    wsb = nc.dram_tensor("wsb", [128, 3 * NCHUNK], F32, kind="ExternalInput").ap()
    lrow = nc.dram_tensor("lrow", [128, NCHUNK], F32, kind="ExternalInput").ap()
    out = nc.dram_tensor("out", [1, 4], F32, kind="ExternalOutput").ap()

    AF = mybir.ActivationFunctionType
    ALU = mybir.AluOpType
    AX = mybir.AxisListType

    with tile.TileContext(nc) as tc:
        with tc.tile_pool(name="consts", bufs=1) as consts, \
             tc.tile_pool(name="xtiles", bufs=3) as xtiles, \
             tc.tile_pool(name="ps", bufs=2, space="PSUM") as psp, \
             tc.tile_pool(name="psq", bufs=1, space="PSUM") as psqp, \
             tc.tile_pool(name="tail", bufs=2) as tailp, \
             tc.tile_pool(name="fin", bufs=1) as finp:
            w_t = consts.tile([128, 3 * NCHUNK], F32)
            nc.sync.dma_start(out=w_t[:], in_=wsb)
            l_t = consts.tile([128, NCHUNK], F32)
            nc.sync.dma_start(out=l_t[:], in_=lrow)

            ps_q = psqp.tile([1, 1], F32)
            qlast = finp.tile([1, 1], F32)
            nm_vec = finp.tile([1, NSLICE], F32)   # -max(logits) per slice
            s_vec = finp.tile([1, NSLICE], F32)    # sum(exp) per slice
            w_vec = finp.tile([1, NSLICE], F32)    # sum(exp * v) per slice

            for s in range(NSLICE):
                ps_k = psp.tile([1, SLICE], F32, tag="ps_k")
                ps_v = psp.tile([1, SLICE], F32, tag="ps_v")
                for t2 in range(NTILE // NSLICE):
                    x_t = xtiles.tile([128, CH_PER_TILE * SLICE], F32, tag="x_t")
                    nc.sync.dma_start(out=x_t[:], in_=xt[s * (NTILE // NSLICE) + t2])
                    for u in range(CH_PER_TILE):
                        c = t2 * CH_PER_TILE + u
                        rhs = x_t[:, u * SLICE:(u + 1) * SLICE]
                        st, sp = (c == 0), (c == NCHUNK - 1)
                        nc.tensor.matmul(ps_k[:], w_t[:, 3 * c + 1:3 * c + 2], rhs,
                                         start=st, stop=sp)
                        nc.tensor.matmul(ps_v[:], w_t[:, 3 * c + 2:3 * c + 3], rhs,
                                         start=st, stop=sp)
                        if s == 0:
                            nc.tensor.matmul(ps_q[:], w_t[:, 3 * c:3 * c + 1],
                                             l_t[:, c:c + 1], start=st, stop=sp)
                if s == 0:
                    nc.scalar.copy(out=qlast[:], in_=ps_q[:])
                # flash-softmax partial stats for this slice of 512 rows
                logits = tailp.tile([1, SLICE], F32, tag="logits")
                nc.vector.tensor_scalar_mul(out=logits[:], in0=ps_k[:],
                                            scalar1=qlast[:])
                nc.vector.tensor_reduce(out=nm_vec[:, s:s + 1], in_=logits[:],
                                        axis=AX.X, op=ALU.max, negate=True)
                e_t = tailp.tile([1, SLICE], F32, tag="e_t")
                nc.scalar.activation(out=e_t[:], in_=logits[:], func=AF.Exp,
                                     bias=nm_vec[:, s:s + 1], scale=1.0,
                                     accum_out=s_vec[:, s:s + 1])
                scr = tailp.tile([1, SLICE], F32, tag="scr")
                nc.vector.tensor_mul(out=scr[:], in0=e_t[:], in1=ps_v[:])
                nc.vector.reduce_sum(out=w_vec[:, s:s + 1], in_=scr[:], axis=AX.X)

            # combine slice stats into per-core stats (still exact flash combine)
            negM = finp.tile([1, 1], F32)
            nc.vector.tensor_reduce(out=negM[:], in_=nm_vec[:], axis=AX.X,
                                    op=ALU.min)
            e2 = finp.tile([1, NSLICE], F32)
            nc.scalar.activation(out=e2[:], in_=nm_vec[:], func=AF.Exp,
                                 bias=negM[:], scale=-1.0)
            outsb = finp.tile([1, 4], F32)
            sc2 = finp.tile([1, NSLICE], F32)
            nc.vector.tensor_mul(out=sc2[:], in0=e2[:], in1=s_vec[:])
            nc.vector.reduce_sum(out=outsb[:, 1:2], in_=sc2[:], axis=AX.X)
            sc3 = finp.tile([1, NSLICE], F32)
            nc.vector.tensor_mul(out=sc3[:], in0=e2[:], in1=w_vec[:])
            nc.vector.reduce_sum(out=outsb[:, 2:3], in_=sc3[:], axis=AX.X)
            nc.vector.tensor_copy(out=outsb[:, 0:1], in_=negM[:])
            nc.vector.memset(outsb[:, 3:4], 0.0)
            nc.sync.dma_start(out=out, in_=outsb[:])
    nc.finalize()
    return nc


def _get_nc():
    global _NC
    if _NC is None:
        _NC = _build_nc()
    return _NC


def _prep_inputs(inputs, in_idxs, weights):
    inputs = np.ascontiguousarray(np.asarray(inputs, dtype=np.float32))
    idx = np.asarray(in_idxs).astype(np.int64)
    w = np.asarray(weights, dtype=np.float32)

    # scatter-add weights into dense [F, 3]: sel @ weights == inputs @ wd
    wd = np.zeros((F, 3), dtype=np.float32)
    np.add.at(wd, idx, w)
    # SBUF layout [128, 3*NCHUNK]: wsb[p, 3c+j] = wd[c*128+p, j]
    wsb = np.ascontiguousarray(
        wd.reshape(NCHUNK, 128, 3).transpose(1, 0, 2).reshape(128, 3 * NCHUNK))
    # last row of inputs, chunked: lrow[p, c] = inputs[R-1, c*128+p]
    lrow = np.ascontiguousarray(inputs[R - 1].reshape(NCHUNK, 128).T)

    # xt[core][tile, p, u*SLICE+col] = inputs[core*RB + s*SLICE + col, c*128 + p]
    # with tile = s*(NTILE/NSLICE)+t2, c = t2*CH_PER_TILE+u
    x6 = inputs.reshape(NCORES, NSLICE, SLICE, NTILE // NSLICE, CH_PER_TILE, 128)
    xt_all = np.ascontiguousarray(x6.transpose(0, 1, 3, 5, 4, 2))
    xt_all = xt_all.reshape(NCORES, NTILE, 128, CH_PER_TILE * SLICE)

    return [{"xt": xt_all[i], "wsb": wsb, "lrow": lrow} for i in range(NCORES)]


def _combine(outs):
    # outs: [NCORES, >=3] rows of (-max_logit, sum_exp, sum_exp_v), fp64 combine
    o = np.asarray(outs, dtype=np.float64)
    m = -o[:, 0]
    s = o[:, 1]
    w = o[:, 2]
    mx = m.max()
    scale = np.exp(m - mx)
    val = (w * scale).sum() / (s * scale).sum()
    return np.array([[1.0 / (1.0 + np.exp(-val))]], dtype=np.float32)


def kernel(inputs, in_idxs, weights):
    nc = _get_nc()
    in_maps = _prep_inputs(inputs, in_idxs, weights)
    res = run_bass_kernel_spmd(nc, in_maps, core_ids=list(range(NCORES)))
    outs = np.stack([res.results[i]["out"][0] for i in range(NCORES)])
    return _combine(outs)


if __name__ == "__main__":
    rng = np.random.default_rng(0)
    inputs = rng.standard_normal((R, F), dtype=np.float32)
    in_idxs = rng.integers(0, F, size=2048)
    weights = rng.standard_normal((2048, 3), dtype=np.float32)
    got = kernel(inputs, in_idxs, weights)
    sel = inputs[:, in_idxs]
    qkv = sel @ weights
    q, k, v = qkv[:, 0], qkv[:, 1], qkv[:, 2]
    logits = q[-1] * k
    a = np.exp(logits - logits.max())
    want = a @ v / a.sum()
    want = 1.0 / (1.0 + np.exp(-want))
    print("got", got, "want", want, "relerr", abs(got[0, 0] - want) / abs(want))


# revision 5
# speedup vs baseline: 1.8905x; 1.8905x over previous
"""Trainium2 Bass kernel for nn_AttentionNeNode (8-core SPMD).

Math being computed (see problem reference):
    sel  = inputs[:, in_idxs]            # [R, L] column gather
    qkv  = sel @ weights                 # [R, 3] -> q, k, v columns
    out  = sigmoid(softmax(q[-1] * k.T) @ v)   # only the LAST row's attention matters

Key transformations:
  1. Column gather + matmul == dense matmul with scattered weights:
         sel @ weights == inputs @ W_dense,
     where W_dense[f] = sum of weights[l] over l with in_idxs[l] == f.
     This turns random column access into a dense streaming read of `inputs`.
  2. Only row R-1's attention is needed, so each core computes k, v for its
     block of rows plus flash-softmax partial stats (max, sum_exp, sum_exp*v);
     host combines 8 triples of scalars (the "unshard" step).
  3. `inputs` is pre-transposed/tiled on host so the contraction dim (F) lands
     on SBUF partitions and every DMA is a fully contiguous 2 MiB block.
"""

import sys

if "/opt/trn_rl_repo" not in sys.path:
    sys.path.insert(0, "/opt/trn_rl_repo")

import numpy as np

import concourse.bacc as bacc
import concourse.tile as tile
from concourse import mybir
from concourse.bass_utils import run_bass_kernel_spmd

R, F = 8192, 4096
NCORES = 8
RB = R // NCORES            # 1024 rows per core
NSLICE = 2                  # row slices per core (fit one PSUM bank each for k and v)
SLICE = RB // NSLICE        # 512
NCHUNK = F // 128           # 32 contraction chunks of 128
CH_PER_TILE = 8             # f-chunks per DMA tile
NTILE = NCHUNK // CH_PER_TILE * NSLICE  # 8 DMA tiles of [128, 4096] = 2 MiB
F32 = mybir.dt.float32
F32R = mybir.dt.float32r

_NC = None


def _build_nc():
    nc = bacc.Bacc("TRN2", target_bir_lowering=False, debug=False)
    xt = nc.dram_tensor("xt", [NTILE, 128, CH_PER_TILE * SLICE], F32R,
                        kind="ExternalInput").ap()
    wsb = nc.dram_tensor("wsb", [128, 3 * NCHUNK], F32R, kind="ExternalInput").ap()
    # last-row chunks duplicated x2: fp32r matmul needs moving free dim >= 2
    lrow = nc.dram_tensor("lrow", [128, 2 * NCHUNK], F32R, kind="ExternalInput").ap()
    out = nc.dram_tensor("out", [1, 4], F32, kind="ExternalOutput").ap()

    AF = mybir.ActivationFunctionType
    ALU = mybir.AluOpType
    AX = mybir.AxisListType

    with tile.TileContext(nc) as tc:
        with tc.tile_pool(name="consts", bufs=1) as consts, \
             tc.tile_pool(name="xtiles", bufs=3) as xtiles, \
             tc.tile_pool(name="ps", bufs=2, space="PSUM") as psp, \
             tc.tile_pool(name="psq", bufs=1, space="PSUM") as psqp, \
             tc.tile_pool(name="tail", bufs=2) as tailp, \
             tc.tile_pool(name="fin", bufs=1) as finp:
            w_t = consts.tile([128, 3 * NCHUNK], F32R)
            nc.sync.dma_start(out=w_t[:], in_=wsb)
            l_t = consts.tile([128, 2 * NCHUNK], F32R)
            nc.sync.dma_start(out=l_t[:], in_=lrow)

            ps_q = psqp.tile([1, 2], F32)
            qlast = finp.tile([1, 1], F32)
            nm_vec = finp.tile([1, NSLICE], F32)   # -max(logits) per slice
            s_vec = finp.tile([1, NSLICE], F32)    # sum(exp) per slice
            w_vec = finp.tile([1, NSLICE], F32)    # sum(exp * v) per slice

            for s in range(NSLICE):
                ps_k = psp.tile([1, SLICE], F32, tag="ps_k")
                ps_v = psp.tile([1, SLICE], F32, tag="ps_v")
                for t2 in range(NTILE // NSLICE):
                    x_t = xtiles.tile([128, CH_PER_TILE * SLICE], F32R, tag="x_t")
                    nc.sync.dma_start(out=x_t[:], in_=xt[s * (NTILE // NSLICE) + t2])
                    for u in range(CH_PER_TILE):
                        c = t2 * CH_PER_TILE + u
                        rhs = x_t[:, u * SLICE:(u + 1) * SLICE]
                        st, sp = (c == 0), (c == NCHUNK - 1)
                        nc.tensor.matmul(ps_k[:], w_t[:, 3 * c + 1:3 * c + 2], rhs,
                                         start=st, stop=sp)
                        nc.tensor.matmul(ps_v[:], w_t[:, 3 * c + 2:3 * c + 3], rhs,
                                         start=st, stop=sp)
                        if s == 0:
                            nc.tensor.matmul(ps_q[:], w_t[:, 3 * c:3 * c + 1],
                                             l_t[:, 2 * c:2 * c + 2],
                                             start=st, stop=sp)
                if s == 0:
                    nc.scalar.copy(out=qlast[:], in_=ps_q[:, 0:1])
                # flash-softmax partial stats for this slice of 512 rows
                logits = tailp.tile([1, SLICE], F32, tag="logits")
                nc.vector.tensor_scalar_mul(out=logits[:], in0=ps_k[:],
                                            scalar1=qlast[:])
                nc.vector.tensor_reduce(out=nm_vec[:, s:s + 1], in_=logits[:],
                                        axis=AX.X, op=ALU.max, negate=True)
                e_t = tailp.tile([1, SLICE], F32, tag="e_t")
                nc.scalar.activation(out=e_t[:], in_=logits[:], func=AF.Exp,
                                     bias=nm_vec[:, s:s + 1], scale=1.0,
                                     accum_out=s_vec[:, s:s + 1])
                scr = tailp.tile([1, SLICE], F32, tag="scr")
                nc.vector.tensor_mul(out=scr[:], in0=e_t[:], in1=ps_v[:])
                nc.vector.reduce_sum(out=w_vec[:, s:s + 1], in_=scr[:], axis=AX.X)

            # combine slice stats into per-core stats (still exact flash combine)
            negM = finp.tile([1, 1], F32)
            nc.vector.tensor_reduce(out=negM[:], in_=nm_vec[:], axis=AX.X,
                                    op=ALU.min)
            e2 = finp.tile([1, NSLICE], F32)
            nc.scalar.activation(out=e2[:], in_=nm_vec[:], func=AF.Exp,
                                 bias=negM[:], scale=-1.0)
            outsb = finp.tile([1, 4], F32)
            sc2 = finp.tile([1, NSLICE], F32)
            nc.vector.tensor_mul(out=sc2[:], in0=e2[:], in1=s_vec[:])
            nc.vector.reduce_sum(out=outsb[:, 1:2], in_=sc2[:], axis=AX.X)
            sc3 = finp.tile([1, NSLICE], F32)
            nc.vector.tensor_mul(out=sc3[:], in0=e2[:], in1=w_vec[:])
            nc.vector.reduce_sum(out=outsb[:, 2:3], in_=sc3[:], axis=AX.X)
            nc.vector.tensor_copy(out=outsb[:, 0:1], in_=negM[:])
            nc.vector.memset(outsb[:, 3:4], 0.0)
            nc.sync.dma_start(out=out, in_=outsb[:])
    nc.finalize()
    return nc


def _get_nc():
    global _NC
    if _NC is None:
        _NC = _build_nc()
    return _NC


def _prep_inputs(inputs, in_idxs, weights):
    inputs = np.ascontiguousarray(np.asarray(inputs, dtype=np.float32))
    idx = np.asarray(in_idxs).astype(np.int64)
    w = np.asarray(weights, dtype=np.float32)

    # scatter-add weights into dense [F, 3]: sel @ weights == inputs @ wd
    wd = np.zeros((F, 3), dtype=np.float32)
    np.add.at(wd, idx, w)
    # SBUF layout [128, 3*NCHUNK]: wsb[p, 3c+j] = wd[c*128+p, j]
    wsb = np.ascontiguousarray(
        wd.reshape(NCHUNK, 128, 3).transpose(1, 0, 2).reshape(128, 3 * NCHUNK))
    # last row of inputs, chunked: lrow[p, c] = inputs[R-1, c*128+p]
    lrow = np.ascontiguousarray(
        np.repeat(inputs[R - 1].reshape(NCHUNK, 128).T, 2, axis=1))

    # xt[core][tile, p, u*SLICE+col] = inputs[core*RB + s*SLICE + col, c*128 + p]
    # with tile = s*(NTILE/NSLICE)+t2, c = t2*CH_PER_TILE+u
    x6 = inputs.reshape(NCORES, NSLICE, SLICE, NTILE // NSLICE, CH_PER_TILE, 128)
    xt_all = np.ascontiguousarray(x6.transpose(0, 1, 3, 5, 4, 2))
    xt_all = xt_all.reshape(NCORES, NTILE, 128, CH_PER_TILE * SLICE)

    return [{"xt": xt_all[i], "wsb": wsb, "lrow": lrow} for i in range(NCORES)]


def _combine(outs):
    # outs: [NCORES, >=3] rows of (-max_logit, sum_exp, sum_exp_v), fp64 combine
    o = np.asarray(outs, dtype=np.float64)
    m = -o[:, 0]
    s = o[:, 1]
    w = o[:, 2]
    mx = m.max()
    scale = np.exp(m - mx)
    val = (w * scale).sum() / (s * scale).sum()
    return np.array([[1.0 / (1.0 + np.exp(-val))]], dtype=np.float32)


def kernel(inputs, in_idxs, weights):
    nc = _get_nc()
    in_maps = _prep_inputs(inputs, in_idxs, weights)
    res = run_bass_kernel_spmd(nc, in_maps, core_ids=list(range(NCORES)))
    outs = np.stack([res.results[i]["out"][0] for i in range(NCORES)])
    return _combine(outs)


if __name__ == "__main__":
    rng = np.random.default_rng(0)
    inputs = rng.standard_normal((R, F), dtype=np.float32)
    in_idxs = rng.integers(0, F, size=2048)
    weights = rng.standard_normal((2048, 3), dtype=np.float32)
    got = kernel(inputs, in_idxs, weights)
    sel = inputs[:, in_idxs]
    qkv = sel @ weights
    q, k, v = qkv[:, 0], qkv[:, 1], qkv[:, 2]
    logits = q[-1] * k
    a = np.exp(logits - logits.max())
    want = a @ v / a.sum()
    want = 1.0 / (1.0 + np.exp(-want))
    print("got", got, "want", want, "relerr", abs(got[0, 0] - want) / abs(want))


# revision 6
# speedup vs baseline: 2.0503x; 1.0845x over previous
"""Trainium2 Bass kernel for nn_AttentionNeNode (8-core SPMD).

Math being computed (see problem reference):
    sel  = inputs[:, in_idxs]            # [R, L] column gather
    qkv  = sel @ weights                 # [R, 3] -> q, k, v columns
    out  = sigmoid(softmax(q[-1] * k.T) @ v)   # only the LAST row's attention matters

Key transformations:
  1. Column gather + matmul == dense matmul with scattered weights:
         sel @ weights == inputs @ W_dense,
     where W_dense[f] = sum of weights[l] over l with in_idxs[l] == f.
     This turns random column access into a dense streaming read of `inputs`.
  2. Only row R-1's attention is needed, so each core computes k, v for its
     block of rows plus flash-softmax partial stats (max, sum_exp, sum_exp*v);
     host combines 8 triples of scalars (the "unshard" step).
  3. `inputs` is pre-transposed/tiled on host so the contraction dim (F) lands
     on SBUF partitions and every DMA is a fully contiguous 2 MiB block.
"""

import sys

if "/opt/trn_rl_repo" not in sys.path:
    sys.path.insert(0, "/opt/trn_rl_repo")

import numpy as np

import concourse.bacc as bacc
import concourse.tile as tile
from concourse import mybir
from concourse.bass_utils import run_bass_kernel_spmd

R, F = 8192, 4096
NCORES = 8
RB = R // NCORES            # 1024 rows per core
NSLICE = 2                  # row slices per core (fit one PSUM bank each for k and v)
SLICE = RB // NSLICE        # 512
NCHUNK = F // 128           # 32 contraction chunks of 128
CH_PER_TILE = 8             # f-chunks per DMA tile
NTILE = NCHUNK // CH_PER_TILE * NSLICE  # 8 DMA tiles of [128, 4096] = 2 MiB
F32 = mybir.dt.float32
F32R = mybir.dt.float32r

_NC = None


def _build_nc():
    nc = bacc.Bacc("TRN2", target_bir_lowering=False, debug=False)
    xt = nc.dram_tensor("xt", [NTILE, 128, CH_PER_TILE * SLICE], F32R,
                        kind="ExternalInput").ap()
    wsb = nc.dram_tensor("wsb", [128, 3 * NCHUNK], F32R, kind="ExternalInput").ap()
    # last-row chunks duplicated x2: fp32r matmul needs moving free dim >= 2
    lrow = nc.dram_tensor("lrow", [128, 2 * NCHUNK], F32R, kind="ExternalInput").ap()
    out = nc.dram_tensor("out", [1, 4], F32, kind="ExternalOutput").ap()

    AF = mybir.ActivationFunctionType
    ALU = mybir.AluOpType
    AX = mybir.AxisListType

    with tile.TileContext(nc) as tc:
        with tc.tile_pool(name="consts", bufs=1) as consts, \
             tc.tile_pool(name="xtiles", bufs=3) as xtiles, \
             tc.tile_pool(name="ps", bufs=2, space="PSUM") as psp, \
             tc.tile_pool(name="psq", bufs=1, space="PSUM") as psqp, \
             tc.tile_pool(name="tail", bufs=2) as tailp, \
             tc.tile_pool(name="fin", bufs=1) as finp:
            w_t = consts.tile([128, 3 * NCHUNK], F32R)
            nc.sync.dma_start(out=w_t[:], in_=wsb)
            l_t = consts.tile([128, 2 * NCHUNK], F32R)
            nc.sync.dma_start(out=l_t[:], in_=lrow)

            ps_q = psqp.tile([1, 2], F32)
            qlast = finp.tile([1, 1], F32)
            nm_vec = finp.tile([1, NSLICE], F32)   # -max(logits) per slice
            s_vec = finp.tile([1, NSLICE], F32)    # sum(exp) per slice
            w_vec = finp.tile([1, NSLICE], F32)    # sum(exp * v) per slice

            for s in range(NSLICE):
                # one m=2 matmul per chunk makes [k; v] rows: stream rhs ONCE
                ps_kv = psp.tile([2, SLICE], F32, tag="ps_kv")
                for t2 in range(NTILE // NSLICE):
                    x_t = xtiles.tile([128, CH_PER_TILE * SLICE], F32R, tag="x_t")
                    nc.sync.dma_start(out=x_t[:], in_=xt[s * (NTILE // NSLICE) + t2])
                    for u in range(CH_PER_TILE):
                        c = t2 * CH_PER_TILE + u
                        rhs = x_t[:, u * SLICE:(u + 1) * SLICE]
                        st, sp = (c == 0), (c == NCHUNK - 1)
                        nc.tensor.matmul(ps_kv[:], w_t[:, 3 * c + 1:3 * c + 3], rhs,
                                         start=st, stop=sp)
                        if s == 0:
                            nc.tensor.matmul(ps_q[:], w_t[:, 3 * c:3 * c + 1],
                                             l_t[:, 2 * c:2 * c + 2],
                                             start=st, stop=sp)
                if s == 0:
                    nc.scalar.copy(out=qlast[:], in_=ps_q[:, 0:1])
                # evacuate PSUM and flatten [2, SLICE] -> [1, 2*SLICE] so k and v
                # share partition 0 (engine operands must start at partition 0)
                kv_sb = tailp.tile([2, SLICE], F32, tag="kv_sb")
                nc.scalar.copy(out=kv_sb[:], in_=ps_kv[:])
                flat = tailp.tile([1, 2 * SLICE], F32, tag="flat")
                nc.sync.dma_start(out=flat[:], in_=kv_sb[:])
                # flash-softmax partial stats for this slice of 512 rows
                logits = tailp.tile([1, SLICE], F32, tag="logits")
                nc.vector.tensor_scalar_mul(out=logits[:], in0=flat[:, 0:SLICE],
                                            scalar1=qlast[:])
                nc.vector.tensor_reduce(out=nm_vec[:, s:s + 1], in_=logits[:],
                                        axis=AX.X, op=ALU.max, negate=True)
                e_t = tailp.tile([1, SLICE], F32, tag="e_t")
                nc.scalar.activation(out=e_t[:], in_=logits[:], func=AF.Exp,
                                     bias=nm_vec[:, s:s + 1], scale=1.0,
                                     accum_out=s_vec[:, s:s + 1])
                scr = tailp.tile([1, SLICE], F32, tag="scr")
                nc.vector.tensor_mul(out=scr[:], in0=e_t[:],
                                     in1=flat[:, SLICE:2 * SLICE])
                nc.vector.reduce_sum(out=w_vec[:, s:s + 1], in_=scr[:], axis=AX.X)

            # combine slice stats into per-core stats (still exact flash combine)
            negM = finp.tile([1, 1], F32)
            nc.vector.tensor_reduce(out=negM[:], in_=nm_vec[:], axis=AX.X,
                                    op=ALU.min)
            e2 = finp.tile([1, NSLICE], F32)
            nc.scalar.activation(out=e2[:], in_=nm_vec[:], func=AF.Exp,
                                 bias=negM[:], scale=-1.0)
            outsb = finp.tile([1, 4], F32)
            sc2 = finp.tile([1, NSLICE], F32)
            nc.vector.tensor_mul(out=sc2[:], in0=e2[:], in1=s_vec[:])
            nc.vector.reduce_sum(out=outsb[:, 1:2], in_=sc2[:], axis=AX.X)
            sc3 = finp.tile([1, NSLICE], F32)
            nc.vector.tensor_mul(out=sc3[:], in0=e2[:], in1=w_vec[:])
            nc.vector.reduce_sum(out=outsb[:, 2:3], in_=sc3[:], axis=AX.X)
            nc.vector.tensor_copy(out=outsb[:, 0:1], in_=negM[:])
            nc.vector.memset(outsb[:, 3:4], 0.0)
            nc.sync.dma_start(out=out, in_=outsb[:])
    nc.finalize()
    return nc


def _get_nc():
    global _NC
    if _NC is None:
        _NC = _build_nc()
    return _NC


def _prep_inputs(inputs, in_idxs, weights):
    inputs = np.ascontiguousarray(np.asarray(inputs, dtype=np.float32))
    idx = np.asarray(in_idxs).astype(np.int64)
    w = np.asarray(weights, dtype=np.float32)

    # scatter-add weights into dense [F, 3]: sel @ weights == inputs @ wd
    wd = np.zeros((F, 3), dtype=np.float32)
    np.add.at(wd, idx, w)
    # SBUF layout [128, 3*NCHUNK]: wsb[p, 3c+j] = wd[c*128+p, j]
    wsb = np.ascontiguousarray(
        wd.reshape(NCHUNK, 128, 3).transpose(1, 0, 2).reshape(128, 3 * NCHUNK))
    # last row of inputs, chunked: lrow[p, c] = inputs[R-1, c*128+p]
    lrow = np.ascontiguousarray(
        np.repeat(inputs[R - 1].reshape(NCHUNK, 128).T, 2, axis=1))

    # xt[core][tile, p, u*SLICE+col] = inputs[core*RB + s*SLICE + col, c*128 + p]
    # with tile = s*(NTILE/NSLICE)+t2, c = t2*CH_PER_TILE+u
    x6 = inputs.reshape(NCORES, NSLICE, SLICE, NTILE // NSLICE, CH_PER_TILE, 128)
    xt_all = np.ascontiguousarray(x6.transpose(0, 1, 3, 5, 4, 2))
    xt_all = xt_all.reshape(NCORES, NTILE, 128, CH_PER_TILE * SLICE)

    return [{"xt": xt_all[i], "wsb": wsb, "lrow": lrow} for i in range(NCORES)]


def _combine(outs):
    # outs: [NCORES, >=3] rows of (-max_logit, sum_exp, sum_exp_v), fp64 combine
    o = np.asarray(outs, dtype=np.float64)
    m = -o[:, 0]
    s = o[:, 1]
    w = o[:, 2]
    mx = m.max()
    scale = np.exp(m - mx)
    val = (w * scale).sum() / (s * scale).sum()
    return np.array([[1.0 / (1.0 + np.exp(-val))]], dtype=np.float32)


def kernel(inputs, in_idxs, weights):
    nc = _get_nc()
    in_maps = _prep_inputs(inputs, in_idxs, weights)
    res = run_bass_kernel_spmd(nc, in_maps, core_ids=list(range(NCORES)))
    outs = np.stack([res.results[i]["out"][0] for i in range(NCORES)])
    return _combine(outs)


if __name__ == "__main__":
    rng = np.random.default_rng(0)
    inputs = rng.standard_normal((R, F), dtype=np.float32)
    in_idxs = rng.integers(0, F, size=2048)
    weights = rng.standard_normal((2048, 3), dtype=np.float32)
    got = kernel(inputs, in_idxs, weights)
    sel = inputs[:, in_idxs]
    qkv = sel @ weights
    q, k, v = qkv[:, 0], qkv[:, 1], qkv[:, 2]
    logits = q[-1] * k
    a = np.exp(logits - logits.max())
    want = a @ v / a.sum()
    want = 1.0 / (1.0 + np.exp(-want))
    print("got", got, "want", want, "relerr", abs(got[0, 0] - want) / abs(want))
